# revision 15
# baseline (speedup 1.0000x reference)
"""Trainium2 Bass kernel for nn_ODE_71743133713072.

Semantics (unrolled from the reference lax.scan):
  out[:, 0]   = lat[:, 0]
  out[:, t+1] = lat[:, t] + dt_eff[t] * f(lat[:, t])   for t = 0..99
                (dt_eff[1] = 0 reproduces the scan's zero-length first gap)
  y = out[:, 100]
  out[:, k+1] = y = y + h * f(y)                        for k = 100..118
where f is the D->U->U->D tanh MLP and all nonzero dt equal h = ts[1]-ts[0]
(linspace; per-step fp32 diffs differ from h by <=1 ulp, far below the bf16
matmul noise floor, so h is folded into W3/b3 on the host).

Sharding: batch 1024 over 8 cores (128 rows/core, exactly the partition
width). Matmuls in bf16 with fp32 PSUM accumulation. Layers 1-2 run
feature-on-partition (weights stationary); layer 3 swaps roles (activations
stationary, h*W3 moving) so its output lands in natural row layout and the
Euler update is a single PSUM+SBUF add. b3*h enters layer 3's accumulation
group via a K=1 ones-row matmul.
"""

import os
import sys
from contextlib import ExitStack

import numpy as np

for _p in ("/opt/trn_rl_repo", "/root/.axon_site/_ro/trn_rl_repo"):
    if os.path.isdir(_p) and _p not in sys.path:
        sys.path.append(_p)

import ml_dtypes  # noqa: E402

B, T_OBS, KPRED, D = 1024, 100, 20, 256
T = T_OBS + KPRED          # 120
NCORES = 8
PB = B // NCORES           # 128 rows per core
P = 128
G = 4                      # time steps per compute group
NG = T_OBS // G            # 25 groups


def _emit(ctx, tc, lat, w3hd, w8d, bpk, br8p, id8d, id32d, ones8d, out, h):
    import concourse.mybir as mybir

    nc = tc.nc
    F32 = mybir.dt.float32
    BF16 = mybir.dt.bfloat16
    FP8 = mybir.dt.float8e4
    AF = mybir.ActivationFunctionType
    ALU = mybir.AluOpType
    DR = mybir.MatmulPerfMode.DoubleRow

    const = ctx.enter_context(tc.tile_pool(name="const", bufs=1))
    w3sb = const.tile([P, 2, D], BF16, tag="w3")
    for kc in range(2):
        nc.sync.dma_start(w3sb[:, kc, :], w3hd[kc * P:(kc + 1) * P, :])
    bsb = const.tile([P, 6], F32, tag="bias")
    nc.sync.dma_start(bsb[:], bpk[:])
    ones8 = const.tile([1, P], FP8, tag="ones8")
    nc.sync.dma_start(ones8[:], ones8d[:])
    # fp8 weights (x8-scaled): [P, 3(w), 2(kc), D]
    w8sb = const.tile([P, 3, 2, D], FP8, tag="w8")
    for wi in range(3):
        for kc in range(2):
            nc.sync.dma_start(w8sb[:, wi, kc, :], w8d[wi, kc * P:(kc + 1) * P, :])
    br8sb = const.tile([1, 2 * D], FP8, tag="br8")
    nc.sync.dma_start(br8sb[:], br8p[:])
    id16 = const.tile([P, P], BF16, tag="id16")
    nc.sync.dma_start(id16[:], id8d[:])
    id32 = const.tile([P, P], F32, tag="id32")
    nc.sync.dma_start(id32[:], id32d[:])

    b1ap = [bsb[:, 0:1], bsb[:, 1:2]]
    b2ap = [bsb[:, 2:3], bsb[:, 3:4]]
    b3hap = [bsb[:, 4:5], bsb[:, 5:6]]

    x32p = ctx.enter_context(tc.tile_pool(name="x32", bufs=4))
    x8p = ctx.enter_context(tc.tile_pool(name="x8", bufs=4))
    xtsbp = ctx.enter_context(tc.tile_pool(name="xtsb", bufs=3))
    hsbp = ctx.enter_context(tc.tile_pool(name="hsb", bufs=4))
    outp = ctx.enter_context(tc.tile_pool(name="outsb", bufs=4))
    chsb = ctx.enter_context(tc.tile_pool(name="chsb", bufs=3))

    xtps = ctx.enter_context(tc.tile_pool(name="xtps", bufs=1, space="PSUM"))
    mmps = ctx.enter_context(tc.tile_pool(name="mmps", bufs=2, space="PSUM"))
    fnps = ctx.enter_context(tc.tile_pool(name="fnps", bufs=2, space="PSUM"))
    chps = ctx.enter_context(tc.tile_pool(name="chps", bufs=1, space="PSUM"))

    h8 = float(h / 8.0)

    def stage_load(g):
        """load + cast + transpose + evac for one group; returns tiles."""
        t0 = g * G
        x32 = x32p.tile([P, G, D], F32, tag="x32")
        nc.sync.dma_start(x32[:], lat[:, t0:t0 + G, :])
        x16 = x8p.tile([P, G, D], BF16, tag="x16")
        nc.gpsimd.tensor_copy(x16[:], x32[:])
        xt = xtps.tile([P, 2, G * P], BF16, tag="xt")
        for tt in range(G):
            for dc in range(2):
                nc.tensor.transpose(
                    xt[:, dc, tt * P:(tt + 1) * P],
                    x16[:, tt, dc * P:(dc + 1) * P], id16[:])
        xts = xtsbp.tile([P, 2, G * P], FP8, tag="xts")
        for dc in range(2):
            nc.vector.tensor_copy(xts[:, dc, :], xt[:, dc, :])
        return x32, xts

    def stage_mlp(pair):
        """L1/L2 for a pair of groups with shared weight loads."""
        h1s = {}
        mm = {}
        for g, (x32, xts) in pair.items():
            mm[g] = mmps.tile([P, 2, G * P], F32, tag="mm", name="mm")
        for mc in range(2):
            for g in pair:
                nc.tensor.matmul(mm[g][:, mc, :],
                                 w8sb[:, 0, :, mc * P:(mc + 1) * P],
                                 pair[g][1][:], start=True, stop=True,
                                 perf_mode=DR)
        for g in pair:
            t = hsbp.tile([P, 2, G * P], FP8, tag="h1")
            for mc in range(2):
                nc.scalar.activation(t[:, mc, :], mm[g][:, mc, :], AF.Tanh,
                                     bias=b1ap[mc], scale=0.125)
            h1s[g] = t
        mm2 = {}
        for g in pair:
            mm2[g] = mmps.tile([P, 2, G * P], F32, tag="mm", name="mm2")
        for mc in range(2):
            for g in pair:
                nc.tensor.matmul(mm2[g][:, mc, :],
                                 w8sb[:, 1, :, mc * P:(mc + 1) * P],
                                 h1s[g][:], start=True, stop=True,
                                 perf_mode=DR)
        h2s = {}
        for g in pair:
            t = hsbp.tile([P, 2, G * P], FP8, tag="h2")
            for mc in range(2):
                nc.scalar.activation(t[:, mc, :], mm2[g][:, mc, :], AF.Tanh,
                                     bias=b2ap[mc], scale=0.125)
            h2s[g] = t
        return h2s

    def stage_out(g, x32, h2s_g):
        """L3 (role-swapped, fp8 DR) + Euler add + store for one group."""
        t0 = g * G
        o32 = outp.tile([P, G, D], F32, tag="o32")
        for half in range(2):
            fn = fnps.tile([P, 2, D], F32, tag="fn")
            # seed each subtile with 8*b3 broadcast (K=1 ones row)
            for i, tt in enumerate((2 * half, 2 * half + 1)):
                nc.tensor.matmul(fn[:, i, :], ones8[:], br8sb[:, 0:D],
                                 start=True, stop=False)
                nc.tensor.matmul(fn[:, i, :],
                                 h2s_g[:, :, tt * P:(tt + 1) * P],
                                 w8sb[:, 2, :, :],
                                 start=False, stop=True, perf_mode=DR)
            if g == 0 and half == 0:
                # t=0: normal Euler step; t=1: dt=0 -> out[:,2] = lat[:,1]
                nc.vector.scalar_tensor_tensor(
                    o32[:, 0, :], fn[:, 0, :], h8, x32[:, 0, :],
                    ALU.mult, ALU.add)
                nc.vector.tensor_copy(o32[:, 1, :], x32[:, 1, :])
            else:
                nc.vector.scalar_tensor_tensor(
                    o32[:, 2 * half:2 * half + 2, :].rearrange("p a b -> p (a b)"),
                    fn.rearrange("p a b -> p (a b)"), h8,
                    x32[:, 2 * half:2 * half + 2, :].rearrange("p a b -> p (a b)"),
                    ALU.mult, ALU.add)
        nc.sync.dma_start(out[:, t0 + 1:t0 + G + 1, :], o32[:])
        return o32

    def do_pair(ga, gb):
        pair = {}
        for g in (ga, gb):
            if g is not None:
                pair[g] = stage_load(g)
        h2s = stage_mlp(pair)
        outs = {}
        for g in pair:
            outs[g] = stage_out(g, pair[g][0], h2s[g])
        return outs

    def chain(o32_24):
        # y0 = out[:, 100] = o32_24[:, 3, :]; chain state transposed fp32.
        y0p = chps.tile([P, 2, P], F32, tag="ch")
        for dc in range(2):
            nc.tensor.transpose(y0p[:, dc, :],
                                o32_24[:, G - 1, dc * P:(dc + 1) * P], id32[:])
        yt = chsb.tile([P, 2, P], F32, tag="yt")
        nc.vector.tensor_copy(yt[:], y0p[:])

        for k in range(T_OBS, T - 1):
            y8 = chsb.tile([P, 2, P], FP8, tag="y8")
            nc.vector.tensor_copy(y8[:], yt[:])
            c1 = chps.tile([P, 2, P], F32, tag="ch")
            for mc in range(2):
                nc.tensor.matmul(c1[:, mc, :],
                                 w8sb[:, 0, :, mc * P:(mc + 1) * P],
                                 y8[:], start=True, stop=True, perf_mode=DR)
            c1s = chsb.tile([P, 2, P], FP8, tag="c1s")
            for mc in range(2):
                nc.scalar.activation(c1s[:, mc, :], c1[:, mc, :], AF.Tanh,
                                     bias=b1ap[mc], scale=0.125)
            c2 = chps.tile([P, 2, P], F32, tag="ch")
            for mc in range(2):
                nc.tensor.matmul(c2[:, mc, :],
                                 w8sb[:, 1, :, mc * P:(mc + 1) * P],
                                 c1s[:], start=True, stop=True, perf_mode=DR)
            c2s = chsb.tile([P, 2, P], BF16, tag="c2s")
            for mc in range(2):
                nc.scalar.activation(c2s[:, mc, :], c2[:, mc, :], AF.Tanh,
                                     bias=b2ap[mc], scale=0.125)
            # L3 in bf16 (w3sb = h*W3); b3*h joins in the update op below.
            c3 = chps.tile([P, 2, P], F32, tag="ch")
            for mc in range(2):
                for kc in range(2):
                    nc.tensor.matmul(c3[:, mc, :],
                                     w3sb[:, kc, mc * P:(mc + 1) * P],
                                     c2s[:, kc, :], start=(kc == 0),
                                     stop=(kc == 1))
            ytn = chsb.tile([P, 2, P], F32, tag="yt")
            for dc in range(2):
                nc.vector.scalar_tensor_tensor(
                    ytn[:, dc, :], c3[:, dc, :], b3hap[dc], yt[:, dc, :],
                    ALU.add, ALU.add)
            yt = ytn

            ynp = chps.tile([P, D], F32, tag="ch")
            for dc in range(2):
                nc.tensor.transpose(ynp[:, dc * P:(dc + 1) * P], yt[:, dc, :], id32[:])
            yns = chsb.tile([P, D], F32, tag="yns")
            nc.vector.tensor_copy(yns[:], ynp[:])
            nc.sync.dma_start(out[:, k + 1, :], yns[:])

    outs = do_pair(NG - 1, NG - 2)
    chain(outs[NG - 1])
    for p in range(0, NG - 2, 2):
        ga = p
        gb = p + 1 if p + 1 < NG - 2 else None
        do_pair(ga, gb)
    nc.sync.dma_start(out[:, 0, :], lat[:, 0, :])


def _build(h):
    import concourse.mybir as mybir
    import concourse.tile as tile
    from concourse import bacc

    F32 = mybir.dt.float32
    BF16 = mybir.dt.bfloat16
    FP8 = mybir.dt.float8e4

    nc = bacc.Bacc("TRN2", target_bir_lowering=False, debug=False,
                   num_devices=NCORES)
    lat = nc.dram_tensor("lat", [PB, T_OBS, D], F32, kind="ExternalInput").ap()
    w3hd = nc.dram_tensor("w3h", [D, D], BF16, kind="ExternalInput").ap()
    w8d = nc.dram_tensor("w8", [3, D, D], FP8, kind="ExternalInput").ap()
    bpk = nc.dram_tensor("bpack", [P, 6], F32, kind="ExternalInput").ap()
    br8p = nc.dram_tensor("brows8", [1, 2 * D], FP8, kind="ExternalInput").ap()
    id8d = nc.dram_tensor("id8", [P, P], BF16, kind="ExternalInput").ap()
    id32d = nc.dram_tensor("id32", [P, P], F32, kind="ExternalInput").ap()
    ones8d = nc.dram_tensor("ones8", [1, P], FP8, kind="ExternalInput").ap()
    out = nc.dram_tensor("out", [PB, T, D], F32, kind="ExternalOutput").ap()

    with tile.TileContext(nc) as tc, ExitStack() as ctx:
        _emit(ctx, tc, lat, w3hd, w8d, bpk, br8p, id8d, id32d, ones8d, out, h)
    nc.compile()
    return nc


def _host_inputs(inputs):
    ts = np.asarray(inputs["time_steps"], np.float32)
    h = float(np.float32(ts[1]) - np.float32(ts[0]))

    bf = ml_dtypes.bfloat16
    f8 = ml_dtypes.float8_e4m3
    w3h = (np.asarray(inputs["W3"], np.float32) * np.float32(h)).astype(bf)
    b1 = np.asarray(inputs["b1"], np.float32)
    b2 = np.asarray(inputs["b2"], np.float32)
    b3h = np.asarray(inputs["b3"], np.float32) * np.float32(h)
    bpack = np.stack([b1[:P], b1[P:], b2[:P], b2[P:], b3h[:P], b3h[P:]],
                     axis=1).astype(np.float32)
    w8 = np.stack([
        (8.0 * np.asarray(inputs["W1"], np.float32)),
        (8.0 * np.asarray(inputs["W2"], np.float32)),
        (8.0 * np.asarray(inputs["W3"], np.float32)),
    ]).astype(f8)
    b3s8 = (8.0 * np.asarray(inputs["b3"], np.float32))
    brows8 = np.concatenate([b3s8, b3s8]).reshape(1, 2 * D).astype(f8)
    id8 = np.eye(P, dtype=np.float32).astype(bf)
    id32 = np.eye(P, dtype=np.float32)
    ones8 = np.ones((1, P), np.float32).astype(f8)

    shared = dict(w3h=w3h, w8=w8, bpack=bpack, brows8=brows8,
                  id8=id8, id32=id32, ones8=ones8)
    return h, shared


_CACHE = {}


def kernel(**inputs):
    from concourse.bass_utils import run_bass_kernel_spmd

    lat_full = np.ascontiguousarray(np.asarray(inputs["latents"], np.float32))
    h, shared = _host_inputs(inputs)

    if h not in _CACHE:
        _CACHE[h] = _build(h)
    nc = _CACHE[h]

    in_maps = []
    for c in range(NCORES):
        m = dict(shared)
        m["lat"] = np.ascontiguousarray(lat_full[c * PB:(c + 1) * PB])
        in_maps.append(m)
    res = run_bass_kernel_spmd(nc, in_maps, list(range(NCORES)))
    outs = [res.results[c]["out"] for c in range(NCORES)]
    return np.concatenate(outs, axis=0)
